# revision 25
# baseline (speedup 1.0000x reference)
"""GCNEncoder (3x GraphConv, D=64) on 8 Trainium2 NeuronCores.

Strategy (transfer-minimized; the axon tunnel at ~70MB/s h2d and ~30MB/s
d2h dominates wall-clock, so every input is shipped once, small, and in
the narrowest dtype that keeps rel-err under tolerance):
  - Host: dedup edges, relabel nodes by in-degree (descending), partition the
    relabeled dst nodes into 128-row blocks dealt round-robin across 8 cores,
    and build a block-ELL structure (per dst-block: K_j neighbor slots per
    node, uniform across cores so a single SPMD program works).
  - Quad-packed gather: the bf16 y table is viewed as [Npad/4, 4*D] (512B
    rows), so a gather token addresses a QUAD of nodes. Only Npad/4 = 25088
    token values exist, which fits one signed-int16 window — no 32768-row
    window splitting, so ELL padding is just max-in-degree per 1024-node
    block (degree-sorted, so near-mean; K_total 1269 vs ideal 1245). A
    shipped int8 selector (src % 4) is expanded ONCE on device into a
    [P, 4*K] masked weight table (is_equal against an iota pattern), after
    which each layer's multiply+reduce treats the gathered [P, 4K, D]
    exactly like plain ELL. Gathers round-robin over 4 SWDGE queues.
  - Per-core inputs, merged into 3 tensors (per-tensor dispatch costs ~4ms):
    pi8 = x shard (int8, per-column scales) | ELL selector; pi16 = gather
    tokens | ELL weights (bf16 bits); pf32 = biases | weight matrices | x
    scales. Output: h3 transposed int8 with a per-column dynamic scale
    (device computes |max| per column, AllReduce-max, rescales in a second
    pass over the stashed pre3.T), plus the scale vector.
  - Linearity: agg @ W_rel == segment_sum(w * (h @ W_rel)[src]), so each layer
    keeps a node-major table y = h @ W_rel in HBM, and the aggregation output
    plus the root term r = h @ W_root + b is already the layer output.
  - Device prologue (layer-1 dense part): per block, upconvert+transpose the
    int8 x block on the PE (dequant scale folded into the PSUM evacuation),
    then y1 = x@W_rel1 and r1 = x@W_root1 + b1 via two matmuls; an AllGather
    builds the bf16 y table. Two more AllGathers rebuild it after layers 1-2.
"""

import os

import numpy as np

P = 128
D = 64
NCORES = 8


# ---------------------------------------------------------------- host prep


def _preprocess(x, edge_index, edge_weight):
    import ml_dtypes

    bf16 = ml_dtypes.bfloat16
    N = x.shape[0]
    src = np.asarray(edge_index[0], dtype=np.int64)
    dst = np.asarray(edge_index[1], dtype=np.int64)
    w = np.asarray(edge_weight, dtype=np.float64)

    # dedup parallel edges (sum weights)
    key = dst * N + src
    ukey, inv = np.unique(key, return_inverse=True)
    uw = np.bincount(inv, weights=w).astype(np.float32)
    udst = (ukey // N).astype(np.int64)
    usrc = (ukey % N).astype(np.int64)

    deg = np.bincount(udst, minlength=N)

    # per-core block count
    B = -(-N // (NCORES * P))  # ceil
    Npad = NCORES * B * P

    # order nodes by degree desc; sorted position t -> orig node order[t]
    order = np.argsort(-deg, kind="stable")
    order_pad = np.concatenate([order, np.full(Npad - N, -1, dtype=np.int64)])

    # sorted block g = j*NCORES + c  ->  core c, slot j
    # new id layout: new = c*B*P + j*P + p  where sorted pos t = g*P + p
    t = np.arange(Npad)
    g = t // P
    p = t % P
    c = g % NCORES
    j = g // NCORES
    newpos_of_sorted = c * (B * P) + j * P + p
    # perm: new id -> orig node (-1 for dummy)
    perm = np.empty(Npad, dtype=np.int64)
    perm[newpos_of_sorted] = order_pad
    # inv_new: orig node -> new id
    sorted_pos = np.empty(N, dtype=np.int64)
    sorted_pos[order] = np.arange(N)
    inv_new = newpos_of_sorted[sorted_pos]

    nd = inv_new[udst]  # new dst id per edge
    ns = inv_new[usrc]  # new src id per edge

    ej_all = (nd % (B * P)) // P
    ep_all = nd % P
    ec_all = nd // (B * P)
    # counts per (core, slot j, partition)
    cnt = np.zeros((NCORES, B, P), dtype=np.int64)
    np.add.at(cnt, (ec_all, ej_all, ep_all), 1)
    K_j = np.maximum(cnt.max(axis=(0, 2)), 1)  # [B] slots per block
    off_j = np.concatenate([[0], np.cumsum(K_j)])
    K_total = int(off_j[-1])

    # rank of each edge within its dst group
    eorder = np.argsort(nd, kind="stable")
    nd_s = nd[eorder]
    ns_s = ns[eorder]
    w_s = uw[eorder]
    first = np.concatenate([[True], nd_s[1:] != nd_s[:-1]])
    gid = np.cumsum(first) - 1
    gstart = np.nonzero(first)[0]
    k_within = np.arange(len(nd_s)) - gstart[gid]

    ec = nd_s // (B * P)
    rem = nd_s % (B * P)
    ej = rem // P
    ep = rem % P
    col = off_j[ej] + k_within

    ell_idx = np.zeros((NCORES, P, K_total), dtype=np.int16)  # quad ids
    ell_sel = np.zeros((NCORES, P, K_total), dtype=np.int8)  # src % 4
    ell_w = np.zeros((NCORES, P, K_total), dtype=np.float32)
    ell_idx[ec, ep, col] = (ns_s // 4).astype(np.int16)
    ell_sel[ec, ep, col] = (ns_s % 4).astype(np.int8)
    ell_w[ec, ep, col] = w_s

    # token-format (wrapped int16) index arrays for dma_gather:
    # per block j: tokens t = k*128 + p over its column range; wrapped
    # [16, ntok/16]. Shipped non-replicated; the device copies the 16-row
    # strip to all eight 16-partition gpsimd groups.
    tok_cum = np.concatenate([[0], np.cumsum(K_j * P)])
    TOK_TOTAL = int(tok_cum[-1])
    idx_tok = np.zeros((NCORES, 16, TOK_TOTAL // 16), dtype=np.int16)
    for jb in range(B):
        K = int(K_j[jb])
        c0 = int(off_j[jb])
        t0 = int(tok_cum[jb])
        ntok = K * P
        blk = ell_idx[:, :, c0 : c0 + K]  # [NCORES, P, K]
        lin = blk.transpose(0, 2, 1).reshape(NCORES, ntok)  # t = k*128+p
        idx_tok[:, :, t0 // 16 : (t0 + ntok) // 16] = lin.reshape(
            NCORES, ntok // 16, 16
        ).transpose(0, 2, 1)

    # per-core x shard, node-major, int8 with per-column scale
    real = perm >= 0
    x_new = np.zeros((Npad, D), dtype=np.float32)
    x_new[real] = np.asarray(x, dtype=np.float32)[perm[real]]
    sx = np.abs(x_new).max(axis=0) / 127.0  # [D]
    sx = np.where(sx == 0, 1.0, sx)
    x_q = np.clip(np.round(x_new / sx[None, :]), -127, 127).astype(np.int8)
    x_arr = np.ascontiguousarray(x_q.reshape(NCORES, B * P, D))

    return dict(
        N=N,
        B=B,
        Npad=Npad,
        perm=perm,
        K_j=K_j,
        off_j=off_j,
        tok_cum=tok_cum,
        TOK_TOTAL=TOK_TOTAL,
        K_total=K_total,
        idx_tok=idx_tok,
        ell_sel=ell_sel,
        ell_w=ell_w.astype(bf16),
        x_arr=x_arr,
        sx=np.ascontiguousarray(sx.astype(np.float32).reshape(D, 1)),
    )


# ---------------------------------------------------------------- bass build


def _build(prep):
    import concourse.bacc as bacc
    import concourse.mybir as mybir
    import concourse.tile as tile
    from concourse.masks import make_identity

    f32 = mybir.dt.float32
    bf = mybir.dt.bfloat16
    i16 = mybir.dt.int16
    i8 = mybir.dt.int8
    B = prep["B"]
    Npad = prep["Npad"]
    K_j = prep["K_j"]
    off_j = prep["off_j"]
    tok_cum = prep["tok_cum"]
    TOK_TOTAL = prep["TOK_TOTAL"]
    K_total = prep["K_total"]
    QD = 4 * D  # quad row width

    nc = bacc.Bacc(
        "TRN2",
        target_bir_lowering=False,
        debug=False,
        num_devices=NCORES,
        num_swdge_queues=4,
    )

    # IO — merged into 3 tensors (per-tensor per-call dispatch cost is ~4ms)
    # pi8:  [P, B*D + K]   = x packed (x[j*128+p, f] -> [p, j*64+f]) | ell_sel
    # pi16: [P, K + K]     = gather tokens (stream F row-major)      | ell_w(bf16 bits)
    # pf32: [P, 385]       = b1|b2|b3 | (W_rel_i ; W_root_i) pairs x3 | sx
    pi8 = nc.dram_tensor("pi8", [P, B * D + K_total], i8, kind="ExternalInput")
    pi16 = nc.dram_tensor("pi16", [P, 2 * K_total], i16, kind="ExternalInput")
    pf32 = nc.dram_tensor("pf32", [P, 385], f32, kind="ExternalInput")
    # h3 is emitted transposed ([D, B*P]) as int8 with a per-column dynamic
    # scale (colmax/127) computed on device and AllReduced across cores.
    out_t = nc.dram_tensor("h3", [D, B * P], i8, kind="ExternalOutput")
    cmax_out = nc.dram_tensor("colmax", [D, 1], f32, kind="ExternalOutput")

    with tile.TileContext(nc) as tc:
        with (
            tc.tile_pool(name="const", bufs=1) as cpool,
            tc.tile_pool(name="dram", bufs=1, space="DRAM") as dpool,
            tc.tile_pool(name="gather", bufs=3) as gpool,
            tc.tile_pool(name="work", bufs=4) as wpool,
            tc.tile_pool(name="psum", bufs=1, space="PSUM") as ppool,
        ):
            # residents
            idx_res = cpool.tile([P, TOK_TOTAL // 16], i16, tag="idx")
            w_bf = cpool.tile([P, K_total], bf, tag="wbf")
            sel_res = cpool.tile([P, K_total], i8, tag="sel")
            w4 = cpool.tile([P, 4 * K_total], bf, tag="w4")
            q4 = cpool.tile([P, 4], bf, tag="q4")
            r_res = cpool.tile([P, B * D], f32, tag="r")
            pre3 = cpool.tile([D, B * P], f32, tag="pre3")
            cmax = cpool.tile([D, B], f32, tag="cmax")
            sx_res = cpool.tile([D, 1], f32, tag="sx")
            scale_res = cpool.tile([D, 1], f32, tag="scale")
            cmr = cpool.tile([D, 1], f32, tag="cmr")
            ident = cpool.tile([P, P], f32, tag="ident")
            wnames = ("W_rel1", "W_root1", "W_rel2", "W_root2", "W_rel3", "W_root3")
            Wt = {k: cpool.tile([D, D], f32, tag=k, name=k) for k in wnames}
            bt = {k: cpool.tile([P, D], f32, tag=k, name=k) for k in ("b1", "b2", "b3")}

            # unpack pi16: tokens (stream F, row-major [128, K]) -> replicate
            # the wrapped [16, T/16] view to all 8 gpsimd groups
            tok_src = pi16.ap()[:, 0:K_total].rearrange("(s r) c -> s r c", r=8)
            for grp in range(8):
                nc.sync.dma_start(
                    out=idx_res[16 * grp : 16 * (grp + 1), :].rearrange(
                        "p (a b) -> p a b", b=K_total
                    ),
                    in_=tok_src,
                )
            nc.sync.dma_start(
                out=w_bf[:], in_=pi16.ap()[:, K_total : 2 * K_total].bitcast(bf)
            )
            # unpack pi8: ell_sel tail (x blocks stream in the prologue)
            nc.sync.dma_start(
                out=sel_res[:], in_=pi8.ap()[:, B * D : B * D + K_total]
            )
            # unpack pf32
            for i, k in enumerate(("b1", "b2", "b3")):
                nc.sync.dma_start(
                    out=bt[k][:], in_=pf32.ap()[:, i * D : (i + 1) * D]
                )
            for i in (1, 2, 3):
                c0 = 192 + (i - 1) * D
                nc.sync.dma_start(
                    out=Wt[f"W_rel{i}"][:], in_=pf32.ap()[0:D, c0 : c0 + D]
                )
                nc.sync.dma_start(
                    out=Wt[f"W_root{i}"][:], in_=pf32.ap()[D : 2 * D, c0 : c0 + D]
                )
            nc.sync.dma_start(out=sx_res[:], in_=pf32.ap()[0:D, 384:385])
            make_identity(nc, ident[:])

            # expand (w, sel) -> w4[p, 4k+s] = w[p,k] * (sel[p,k]==s)
            for s in range(4):
                nc.vector.memset(q4[:, s : s + 1], float(s))
            w4v = w4[:].rearrange("p (k s) -> p k s", s=4)
            nc.vector.tensor_copy(
                out=w4v,
                in_=sel_res[:].unsqueeze(-1).to_broadcast([P, K_total, 4]),
            )
            nc.vector.tensor_tensor(
                out=w4v,
                in0=w4v,
                in1=q4[:].unsqueeze(1).to_broadcast([P, K_total, 4]),
                op=mybir.AluOpType.is_equal,
            )
            nc.vector.tensor_tensor(
                out=w4v,
                in0=w4v,
                in1=w_bf[:].unsqueeze(-1).to_broadcast([P, K_total, 4]),
                op=mybir.AluOpType.mult,
            )

            # DRAM: y table (bf16, viewed as 512B quad rows) + own staging
            table2 = dpool.tile([Npad // 4, QD], bf, tag="table")
            y_own = dpool.tile([B * P, D], bf, tag="yown")
            cm_own = dpool.tile([D, 1], f32, tag="cmown")
            cm_red = dpool.tile([D, 1], f32, tag="cmred")

            # ---------------- prologue: y1 = x@W_rel1, r1 = x@W_root1 + b1
            # x arrives int8 (per-column scales sx); the raw integers are
            # upconverted node-major, transposed on the PE, and the dequant
            # scale is applied per-partition during the PSUM evacuation.
            for jb in range(B):
                xb = wpool.tile([P, D], i8, tag="xb")
                nc.sync.dma_start(
                    out=xb[:], in_=pi8.ap()[:, jb * D : (jb + 1) * D]
                )
                xf = wpool.tile([P, D], f32, tag="xf")
                nc.scalar.activation(
                    out=xf[:], in_=xb[:], func=mybir.ActivationFunctionType.Copy
                )
                xTp = ppool.tile([D, P], f32, tag="preT", bufs=2)
                nc.tensor.transpose(out=xTp[:], in_=xf[:], identity=ident[:])
                xT = wpool.tile([D, P], f32, tag="hT")
                nc.scalar.activation(
                    out=xT[:],
                    in_=xTp[:],
                    func=mybir.ActivationFunctionType.Copy,
                    scale=sx_res[:],
                )
                yp = ppool.tile([P, D], f32, tag="ynp", bufs=2)
                nc.tensor.matmul(
                    out=yp[:], lhsT=xT[:], rhs=Wt["W_rel1"][:], start=True, stop=True
                )
                ys = wpool.tile([P, D], bf, tag="ysb")
                nc.scalar.activation(
                    out=ys[:], in_=yp[:], func=mybir.ActivationFunctionType.Copy
                )
                nc.sync.dma_start(out=y_own[jb * P : (jb + 1) * P, :], in_=ys[:])
                rp = ppool.tile([P, D], f32, tag="rnp", bufs=2)
                nc.tensor.matmul(
                    out=rp[:], lhsT=xT[:], rhs=Wt["W_root1"][:], start=True, stop=True
                )
                nc.vector.tensor_add(
                    out=r_res[:, jb * D : (jb + 1) * D], in0=rp[:], in1=bt["b1"][:]
                )

            nc.gpsimd.collective_compute(
                "AllGather",
                mybir.AluOpType.bypass,
                replica_groups=[list(range(NCORES))],
                ins=[y_own[:].opt()],
                outs=[table2[:].opt()],
            )

            # ---------------- 3 gather/aggregate layers
            for layer in (1, 2, 3):
                W_rel_nxt = Wt[f"W_rel{layer + 1}"] if layer < 3 else None
                W_root_nxt = Wt[f"W_root{layer + 1}"] if layer < 3 else None
                b_nxt = bt[f"b{layer + 1}"] if layer < 3 else None

                for jb in range(B):
                    K = int(K_j[jb])
                    off = int(off_j[jb])
                    ntok = K * P
                    t0 = int(tok_cum[jb])
                    g = gpool.tile([P, K * QD], bf, tag="g")
                    nc.gpsimd.dma_gather(
                        out_ap=g[:].rearrange("p (c e) -> p c e", e=QD),
                        in_ap=table2[:],
                        idxs_ap=idx_res[:, t0 // 16 : (t0 + ntok) // 16],
                        num_idxs=ntok,
                        num_idxs_reg=ntok,
                        elem_size=QD,
                        single_packet=False,
                        queue_num=jb % 4,
                    )
                    # g *= w4 (broadcast along feature dim); slots are 4K wide
                    g3 = g[:].rearrange("p (k f) -> p k f", f=D)
                    wb = (
                        w4[:, 4 * off : 4 * (off + K)]
                        .unsqueeze(-1)
                        .to_broadcast([P, 4 * K, D])
                    )
                    nc.vector.tensor_tensor(
                        out=g3, in0=g3, in1=wb, op=mybir.AluOpType.mult
                    )
                    # agg[p, f] = sum_k g[p, k, f]
                    agg = wpool.tile([P, D], f32, tag="agg")
                    gT = g[:].rearrange("p (k f) -> p f k", f=D)
                    nc.vector.reduce_sum(
                        out=agg[:], in_=gT, axis=mybir.AxisListType.X
                    )

                    # pre = agg + r
                    pre = wpool.tile([P, D], f32, tag="pre")
                    nc.vector.tensor_add(
                        out=pre[:],
                        in0=agg[:],
                        in1=r_res[:, jb * D : (jb + 1) * D],
                    )

                    if layer == 3:
                        # stash pre.T and its per-column |max| partial; the
                        # int8 emit happens after the cross-core max reduce
                        preT = ppool.tile([D, P], f32, tag="preT", bufs=2)
                        nc.tensor.transpose(
                            out=preT[:], in_=pre[:], identity=ident[:]
                        )
                        nc.scalar.activation(
                            out=pre3[:, jb * P : (jb + 1) * P],
                            in_=preT[:],
                            func=mybir.ActivationFunctionType.Copy,
                        )
                        nc.vector.reduce_max(
                            out=cmax[:, jb : jb + 1],
                            in_=pre3[:, jb * P : (jb + 1) * P],
                            axis=mybir.AxisListType.X,
                            apply_absolute_value=True,
                        )
                        continue
                    # hT = relu(pre).T  via PE transpose + ACT evacuation
                    preT = ppool.tile([D, P], f32, tag="preT", bufs=2)
                    nc.tensor.transpose(out=preT[:], in_=pre[:], identity=ident[:])
                    hT = wpool.tile([D, P], f32, tag="hT")
                    nc.scalar.activation(
                        out=hT[:], in_=preT[:], func=mybir.ActivationFunctionType.Relu
                    )
                    # y_next = h @ W_rel (node-major direct: lhsT = hT)
                    ynp = ppool.tile([P, D], f32, tag="ynp", bufs=2)
                    nc.tensor.matmul(
                        out=ynp[:], lhsT=hT[:], rhs=W_rel_nxt[:], start=True, stop=True
                    )
                    ysb = wpool.tile([P, D], bf, tag="ysb")
                    nc.scalar.activation(
                        out=ysb[:], in_=ynp[:], func=mybir.ActivationFunctionType.Copy
                    )
                    nc.sync.dma_start(
                        out=y_own[jb * P : (jb + 1) * P, :], in_=ysb[:]
                    )
                    # r_next = h @ W_root + b (bias via DVE during PSUM evac)
                    rnp = ppool.tile([P, D], f32, tag="rnp", bufs=2)
                    nc.tensor.matmul(
                        out=rnp[:], lhsT=hT[:], rhs=W_root_nxt[:], start=True, stop=True
                    )
                    nc.vector.tensor_add(
                        out=r_res[:, jb * D : (jb + 1) * D],
                        in0=rnp[:],
                        in1=b_nxt[:],
                    )

                if layer < 3:
                    nc.gpsimd.collective_compute(
                        "AllGather",
                        mybir.AluOpType.bypass,
                        replica_groups=[list(range(NCORES))],
                        ins=[y_own[:].opt()],
                        outs=[table2[:].opt()],
                    )

            # ---------------- epilogue: global colmax -> int8 emit
            # cm1 = (per-core colmax)/127; AllReduce-max; emit scale = 1/cm1.
            # The host dequant scale is then exactly the shipped colmax value.
            cm1 = wpool.tile([D, 1], f32, tag="cm1")
            nc.vector.reduce_max(
                out=cm1[:], in_=cmax[:], axis=mybir.AxisListType.X
            )
            nc.vector.tensor_scalar_mul(out=cm1[:], in0=cm1[:], scalar1=1.0 / 127.0)
            nc.sync.dma_start(out=cm_own[:], in_=cm1[:])
            nc.gpsimd.collective_compute(
                "AllReduce",
                mybir.AluOpType.max,
                replica_groups=[list(range(NCORES))],
                ins=[cm_own[:].opt()],
                outs=[cm_red[:].opt()],
            )
            nc.sync.dma_start(out=cmr[:], in_=cm_red[:])
            nc.sync.dma_start(out=cmax_out.ap(), in_=cm_red[:])
            nc.vector.reciprocal(out=scale_res[:], in_=cmr[:])
            for jb in range(B):
                obi = wpool.tile([D, P], i8, tag="obi")
                nc.scalar.activation(
                    out=obi[:],
                    in_=pre3[:, jb * P : (jb + 1) * P],
                    func=mybir.ActivationFunctionType.Copy,
                    scale=scale_res[:],
                )
                nc.sync.dma_start(
                    out=out_t.ap()[:, jb * P : (jb + 1) * P], in_=obi[:]
                )

    nc.compile()
    return nc


# ---------------------------------------------------------------- entry


def _prep_and_build(inputs):
    prep = _preprocess(inputs["x"], inputs["edge_index"], inputs["edge_weight"])
    nc = _build(prep)
    B = prep["B"]
    K = prep["K_total"]

    # pf32 (identical on every core): b1|b2|b3 | W pairs | sx
    pf32 = np.zeros((P, 385), dtype=np.float32)
    for i in (1, 2, 3):
        b = np.asarray(inputs[f"b_rel{i}"], dtype=np.float32)
        pf32[:, (i - 1) * D : i * D] = b[None, :]
        pf32[0:D, 192 + (i - 1) * D : 192 + i * D] = np.asarray(
            inputs[f"W_rel{i}"], dtype=np.float32
        )
        pf32[D : 2 * D, 192 + (i - 1) * D : 192 + i * D] = np.asarray(
            inputs[f"W_root{i}"], dtype=np.float32
        )
    pf32[0:D, 384:385] = prep["sx"]

    in_maps = []
    for c in range(NCORES):
        x_pb = (
            prep["x_arr"][c].reshape(B, P, D).transpose(1, 0, 2).reshape(P, B * D)
        )
        pi8 = np.concatenate([x_pb, prep["ell_sel"][c]], axis=1)
        toks = prep["idx_tok"][c].reshape(-1).reshape(P, K)
        pi16 = np.concatenate([toks, prep["ell_w"][c].view(np.int16)], axis=1)
        in_maps.append(
            {
                "pi8": np.ascontiguousarray(pi8),
                "pi16": np.ascontiguousarray(pi16),
                "pf32": pf32,
            }
        )
    return prep, nc, in_maps


def _reassemble(prep, core_outs, core_cmax):
    N = prep["N"]
    B = prep["B"]
    perm = prep["perm"]
    out = np.zeros((N, D), dtype=np.float32)
    for c in range(NCORES):
        # h3 arrives transposed [D, B*P] int8; colmax is already the
        # dequant scale (global |max|/127)
        sc = core_cmax[c].reshape(D).astype(np.float32)
        h = core_outs[c].astype(np.float32).T * sc[None, :]
        pr = perm[c * B * P : (c + 1) * B * P]
        real = pr >= 0
        out[pr[real]] = h[real]
    return out


def kernel(**inputs) -> np.ndarray:
    from concourse.bass_utils import run_bass_kernel_spmd

    prep, nc, in_maps = _prep_and_build(inputs)
    res = run_bass_kernel_spmd(
        nc,
        in_maps,
        core_ids=list(range(NCORES)),
        trace=bool(int(os.environ.get("GCN_TRACE", "0"))),
    )
    kernel.last_results = res
    kernel.last_nc = nc
    kernel.last_in_maps = in_maps
    return _reassemble(
        prep,
        [res.results[c]["h3"] for c in range(NCORES)],
        [res.results[c]["colmax"] for c in range(NCORES)],
    )


if __name__ == "__main__":
    import reference

    inputs = {k: np.asarray(v) for k, v in reference.setup_inputs().items()}
    expected = np.asarray(reference.reference(**inputs))
    actual = kernel(**inputs)
    err = np.abs(actual - expected).max() / (np.abs(expected).max() + 1e-9)
    rel = np.linalg.norm(actual - expected) / (np.linalg.norm(expected) + 1e-30)
    print("max-abs-rel:", err, " fro-rel:", rel)


# revision 27
# speedup vs baseline: 1.0884x; 1.0884x over previous
"""GCNEncoder (3x GraphConv, D=64) on 8 Trainium2 NeuronCores.

Strategy (transfer-minimized; the axon tunnel at ~70MB/s h2d and ~30MB/s
d2h dominates wall-clock, so every input is shipped once, small, and in
the narrowest dtype that keeps rel-err under tolerance):
  - Host: dedup edges, relabel nodes by in-degree (descending), partition the
    relabeled dst nodes into 128-row blocks dealt round-robin across 8 cores,
    and build a block-ELL structure (per dst-block: K_j neighbor slots per
    node, uniform across cores so a single SPMD program works).
  - Quad-packed gather: the bf16 y table is viewed as [Npad/4, 4*D] (512B
    rows), so a gather token addresses a QUAD of nodes. Only Npad/4 = 25088
    token values exist, which fits one signed-int16 window — no 32768-row
    window splitting, so ELL padding is just max-in-degree per 1024-node
    block (degree-sorted, so near-mean; K_total 1269 vs ideal 1245). A
    shipped int8 selector (src % 4) is expanded ONCE on device into a
    [P, 4*K] masked weight table (is_equal against an iota pattern), after
    which each layer's multiply+reduce treats the gathered [P, 4K, D]
    exactly like plain ELL. Gathers round-robin over 4 SWDGE queues.
  - Per-core inputs, merged into 3 tensors (per-tensor dispatch costs ~4ms):
    pi8 = x shard (int8, per-column scales) | ELL selector; pi16 = gather
    tokens | ELL weights (bf16 bits); pf32 = biases | weight matrices | x
    scales. Output: h3 transposed int8 with a per-column dynamic scale
    (device computes |max| per column, AllReduce-max, rescales in a second
    pass over the stashed pre3.T), plus the scale vector.
  - Linearity: agg @ W_rel == segment_sum(w * (h @ W_rel)[src]), so each layer
    keeps a node-major table y = h @ W_rel in HBM, and the aggregation output
    plus the root term r = h @ W_root + b is already the layer output.
  - Device prologue (layer-1 dense part): per block, upconvert+transpose the
    int8 x block on the PE (dequant scale folded into the PSUM evacuation),
    then y1 = x@W_rel1 and r1 = x@W_root1 + b1 via two matmuls; an AllGather
    builds the bf16 y table. Two more AllGathers rebuild it after layers 1-2.
"""

import os

import numpy as np

P = 128
D = 64
NCORES = 8


# ---------------------------------------------------------------- host prep


def _preprocess(x, edge_index, edge_weight):
    import ml_dtypes

    bf16 = ml_dtypes.bfloat16
    N = x.shape[0]
    src = np.asarray(edge_index[0], dtype=np.int64)
    dst = np.asarray(edge_index[1], dtype=np.int64)
    w = np.asarray(edge_weight, dtype=np.float64)

    # dedup parallel edges (sum weights)
    key = dst * N + src
    ukey, inv = np.unique(key, return_inverse=True)
    uw = np.bincount(inv, weights=w).astype(np.float32)
    udst = (ukey // N).astype(np.int64)
    usrc = (ukey % N).astype(np.int64)

    deg = np.bincount(udst, minlength=N)

    # per-core block count
    B = -(-N // (NCORES * P))  # ceil
    Npad = NCORES * B * P

    # order nodes by degree desc; sorted position t -> orig node order[t]
    order = np.argsort(-deg, kind="stable")
    order_pad = np.concatenate([order, np.full(Npad - N, -1, dtype=np.int64)])

    # sorted block g = j*NCORES + c  ->  core c, slot j
    # new id layout: new = c*B*P + j*P + p  where sorted pos t = g*P + p
    t = np.arange(Npad)
    g = t // P
    p = t % P
    c = g % NCORES
    j = g // NCORES
    newpos_of_sorted = c * (B * P) + j * P + p
    # perm: new id -> orig node (-1 for dummy)
    perm = np.empty(Npad, dtype=np.int64)
    perm[newpos_of_sorted] = order_pad
    # inv_new: orig node -> new id
    sorted_pos = np.empty(N, dtype=np.int64)
    sorted_pos[order] = np.arange(N)
    inv_new = newpos_of_sorted[sorted_pos]

    nd = inv_new[udst]  # new dst id per edge
    ns = inv_new[usrc]  # new src id per edge

    ej_all = (nd % (B * P)) // P
    ep_all = nd % P
    ec_all = nd // (B * P)
    # counts per (core, slot j, partition)
    cnt = np.zeros((NCORES, B, P), dtype=np.int64)
    np.add.at(cnt, (ec_all, ej_all, ep_all), 1)
    K_j = np.maximum(cnt.max(axis=(0, 2)), 1)  # [B] slots per block
    off_j = np.concatenate([[0], np.cumsum(K_j)])
    K_total = int(off_j[-1])

    # rank of each edge within its dst group
    eorder = np.argsort(nd, kind="stable")
    nd_s = nd[eorder]
    ns_s = ns[eorder]
    w_s = uw[eorder]
    first = np.concatenate([[True], nd_s[1:] != nd_s[:-1]])
    gid = np.cumsum(first) - 1
    gstart = np.nonzero(first)[0]
    k_within = np.arange(len(nd_s)) - gstart[gid]

    ec = nd_s // (B * P)
    rem = nd_s % (B * P)
    ej = rem // P
    ep = rem % P
    col = off_j[ej] + k_within

    ell_idx = np.zeros((NCORES, P, K_total), dtype=np.int16)  # quad ids
    ell_sel = np.zeros((NCORES, P, K_total), dtype=np.int8)  # src % 4
    ell_w = np.zeros((NCORES, P, K_total), dtype=np.float32)
    ell_idx[ec, ep, col] = (ns_s // 4).astype(np.int16)
    ell_sel[ec, ep, col] = (ns_s % 4).astype(np.int8)
    ell_w[ec, ep, col] = w_s

    # token-format (wrapped int16) index arrays for dma_gather:
    # per block j: tokens t = k*128 + p over its column range; wrapped
    # [16, ntok/16]. Shipped non-replicated; the device copies the 16-row
    # strip to all eight 16-partition gpsimd groups.
    tok_cum = np.concatenate([[0], np.cumsum(K_j * P)])
    TOK_TOTAL = int(tok_cum[-1])
    idx_tok = np.zeros((NCORES, 16, TOK_TOTAL // 16), dtype=np.int16)
    for jb in range(B):
        K = int(K_j[jb])
        c0 = int(off_j[jb])
        t0 = int(tok_cum[jb])
        ntok = K * P
        blk = ell_idx[:, :, c0 : c0 + K]  # [NCORES, P, K]
        lin = blk.transpose(0, 2, 1).reshape(NCORES, ntok)  # t = k*128+p
        idx_tok[:, :, t0 // 16 : (t0 + ntok) // 16] = lin.reshape(
            NCORES, ntok // 16, 16
        ).transpose(0, 2, 1)

    # per-core x shard, node-major, int8 with per-column scale
    real = perm >= 0
    x_new = np.zeros((Npad, D), dtype=np.float32)
    x_new[real] = np.asarray(x, dtype=np.float32)[perm[real]]
    sx = np.abs(x_new).max(axis=0) / 127.0  # [D]
    sx = np.where(sx == 0, 1.0, sx)
    x_q = np.clip(np.round(x_new / sx[None, :]), -127, 127).astype(np.int8)
    x_arr = np.ascontiguousarray(x_q.reshape(NCORES, B * P, D))

    # pack selectors 4-per-byte (2 bits each)
    M4 = -(-K_total // 4) * 4
    sel_pad = np.zeros((NCORES, P, M4), dtype=np.int8)
    sel_pad[:, :, :K_total] = ell_sel
    s4 = sel_pad.reshape(NCORES, P, M4 // 4, 4)
    sel_pk = (
        s4[..., 0] | (s4[..., 1] << 2) | (s4[..., 2] << 4) | (s4[..., 3] << 6)
    ).astype(np.int8)

    return dict(
        N=N,
        B=B,
        Npad=Npad,
        perm=perm,
        K_j=K_j,
        off_j=off_j,
        tok_cum=tok_cum,
        TOK_TOTAL=TOK_TOTAL,
        K_total=K_total,
        idx_tok=idx_tok,
        sel_pk=sel_pk,
        ell_w=ell_w.astype(bf16),
        x_arr=x_arr,
        sx=np.ascontiguousarray(sx.astype(np.float32).reshape(D, 1)),
    )


# ---------------------------------------------------------------- bass build


def _build(prep):
    import concourse.bacc as bacc
    import concourse.mybir as mybir
    import concourse.tile as tile
    from concourse.masks import make_identity

    f32 = mybir.dt.float32
    bf = mybir.dt.bfloat16
    i16 = mybir.dt.int16
    i8 = mybir.dt.int8
    B = prep["B"]
    Npad = prep["Npad"]
    K_j = prep["K_j"]
    off_j = prep["off_j"]
    tok_cum = prep["tok_cum"]
    TOK_TOTAL = prep["TOK_TOTAL"]
    K_total = prep["K_total"]
    QD = 4 * D  # quad row width

    nc = bacc.Bacc(
        "TRN2",
        target_bir_lowering=False,
        debug=False,
        num_devices=NCORES,
        num_swdge_queues=4,
    )

    # IO — merged into 3 tensors (per-tensor per-call dispatch cost is ~4ms)
    # pi8:  [P, B*D + K]   = x packed (x[j*128+p, f] -> [p, j*64+f]) | ell_sel
    # pi16: [P, K + K]     = gather tokens (stream F row-major)      | ell_w(bf16 bits)
    # pf32: [P, 385]       = b1|b2|b3 | (W_rel_i ; W_root_i) pairs x3 | sx
    M4 = -(-K_total // 4)  # packed selector bytes per partition
    pi8 = nc.dram_tensor("pi8", [P, B * D + M4], i8, kind="ExternalInput")
    pi16 = nc.dram_tensor("pi16", [P, 2 * K_total], i16, kind="ExternalInput")
    pf32 = nc.dram_tensor("pf32", [P, 385], f32, kind="ExternalInput")
    # h3 is emitted transposed ([D, B*P]) as int8 with a per-column dynamic
    # scale (colmax/127) computed on device and AllReduced across cores.
    out_t = nc.dram_tensor("h3", [D, B * P], i8, kind="ExternalOutput")
    cmax_out = nc.dram_tensor("colmax", [D, 1], f32, kind="ExternalOutput")

    with tile.TileContext(nc) as tc:
        with (
            tc.tile_pool(name="const", bufs=1) as cpool,
            tc.tile_pool(name="dram", bufs=1, space="DRAM") as dpool,
            tc.tile_pool(name="gather", bufs=3) as gpool,
            tc.tile_pool(name="work", bufs=4) as wpool,
            tc.tile_pool(name="psum", bufs=1, space="PSUM") as ppool,
        ):
            # residents
            idx_res = cpool.tile([P, TOK_TOTAL // 16], i16, tag="idx")
            w_bf = cpool.tile([P, K_total], bf, tag="wbf")
            spu = cpool.tile([P, M4], i8, tag="selpk")
            sel4i = cpool.tile([P, 4 * M4], i8, tag="sel4")
            sh4 = cpool.tile([P, 4], i8, tag="sh4")
            w4 = cpool.tile([P, 4 * K_total], bf, tag="w4")
            q4 = cpool.tile([P, 4], bf, tag="q4")
            r_res = cpool.tile([P, B * D], f32, tag="r")
            pre3 = cpool.tile([D, B * P], f32, tag="pre3")
            cmax = cpool.tile([D, B], f32, tag="cmax")
            sx_res = cpool.tile([D, 1], f32, tag="sx")
            scale_res = cpool.tile([D, 1], f32, tag="scale")
            cmr = cpool.tile([D, 1], f32, tag="cmr")
            ident = cpool.tile([P, P], f32, tag="ident")
            wnames = ("W_rel1", "W_root1", "W_rel2", "W_root2", "W_rel3", "W_root3")
            Wt = {k: cpool.tile([D, D], f32, tag=k, name=k) for k in wnames}
            bt = {k: cpool.tile([P, D], f32, tag=k, name=k) for k in ("b1", "b2", "b3")}

            # unpack pi16: tokens (stream F, row-major [128, K]) -> replicate
            # the wrapped [16, T/16] view to all 8 gpsimd groups
            tok_src = pi16.ap()[:, 0:K_total].rearrange("(s r) c -> s r c", r=8)
            for grp in range(8):
                nc.sync.dma_start(
                    out=idx_res[16 * grp : 16 * (grp + 1), :].rearrange(
                        "p (a b) -> p a b", b=K_total
                    ),
                    in_=tok_src,
                )
            nc.sync.dma_start(
                out=w_bf[:], in_=pi16.ap()[:, K_total : 2 * K_total].bitcast(bf)
            )
            # unpack pi8: packed selector tail (x blocks stream in the prologue)
            nc.sync.dma_start(out=spu[:], in_=pi8.ap()[:, B * D : B * D + M4])
            # pf32 ships real data on core 0 only (zeros elsewhere compress
            # ~3x in the tunnel); AllReduce(add) broadcasts it to every core
            pfs = dpool.tile([P, 385], f32, tag="pfstage")
            pfr = dpool.tile([P, 385], f32, tag="pfred")
            nc.sync.dma_start(out=pfs[:], in_=pf32.ap())
            nc.gpsimd.collective_compute(
                "AllReduce",
                mybir.AluOpType.add,
                replica_groups=[list(range(NCORES))],
                ins=[pfs[:].opt()],
                outs=[pfr[:].opt()],
            )
            for i, k in enumerate(("b1", "b2", "b3")):
                nc.sync.dma_start(out=bt[k][:], in_=pfr[:, i * D : (i + 1) * D])
            for i in (1, 2, 3):
                c0 = 192 + (i - 1) * D
                nc.sync.dma_start(
                    out=Wt[f"W_rel{i}"][:], in_=pfr[0:D, c0 : c0 + D]
                )
                nc.sync.dma_start(
                    out=Wt[f"W_root{i}"][:], in_=pfr[D : 2 * D, c0 : c0 + D]
                )
            nc.sync.dma_start(out=sx_res[:], in_=pfr[0:D, 384:385])
            make_identity(nc, ident[:])

            # unpack 2-bit selectors: sel4i[p, 4m+j] = (spu[p,m] >> 2j) & 3
            for j in range(4):
                nc.vector.memset(sh4[:, j : j + 1], 2 * j)
            nc.vector.tensor_tensor(
                out=sel4i[:].rearrange("p (m j) -> p m j", j=4),
                in0=spu[:].unsqueeze(-1).to_broadcast([P, M4, 4]),
                in1=sh4[:].unsqueeze(1).to_broadcast([P, M4, 4]),
                op=mybir.AluOpType.logical_shift_right,
            )
            nc.vector.tensor_scalar(
                out=sel4i[:],
                in0=sel4i[:],
                scalar1=3,
                scalar2=None,
                op0=mybir.AluOpType.bitwise_and,
            )
            # expand (w, sel) -> w4[p, 4k+s] = w[p,k] * (sel[p,k]==s)
            for s in range(4):
                nc.vector.memset(q4[:, s : s + 1], float(s))
            w4v = w4[:].rearrange("p (k s) -> p k s", s=4)
            nc.vector.tensor_copy(
                out=w4v,
                in_=sel4i[:, 0:K_total].unsqueeze(-1).to_broadcast([P, K_total, 4]),
            )
            nc.vector.tensor_tensor(
                out=w4v,
                in0=w4v,
                in1=q4[:].unsqueeze(1).to_broadcast([P, K_total, 4]),
                op=mybir.AluOpType.is_equal,
            )
            nc.vector.tensor_tensor(
                out=w4v,
                in0=w4v,
                in1=w_bf[:].unsqueeze(-1).to_broadcast([P, K_total, 4]),
                op=mybir.AluOpType.mult,
            )

            # DRAM: y table (bf16, viewed as 512B quad rows) + own staging
            table2 = dpool.tile([Npad // 4, QD], bf, tag="table")
            y_own = dpool.tile([B * P, D], bf, tag="yown")
            cm_own = dpool.tile([D, 1], f32, tag="cmown")
            cm_red = dpool.tile([D, 1], f32, tag="cmred")

            # ---------------- prologue: y1 = x@W_rel1, r1 = x@W_root1 + b1
            # x arrives int8 (per-column scales sx); the raw integers are
            # upconverted node-major, transposed on the PE, and the dequant
            # scale is applied per-partition during the PSUM evacuation.
            for jb in range(B):
                xb = wpool.tile([P, D], i8, tag="xb")
                nc.sync.dma_start(
                    out=xb[:], in_=pi8.ap()[:, jb * D : (jb + 1) * D]
                )
                xf = wpool.tile([P, D], f32, tag="xf")
                nc.scalar.activation(
                    out=xf[:], in_=xb[:], func=mybir.ActivationFunctionType.Copy
                )
                xTp = ppool.tile([D, P], f32, tag="preT", bufs=2)
                nc.tensor.transpose(out=xTp[:], in_=xf[:], identity=ident[:])
                xT = wpool.tile([D, P], f32, tag="hT")
                nc.scalar.activation(
                    out=xT[:],
                    in_=xTp[:],
                    func=mybir.ActivationFunctionType.Copy,
                    scale=sx_res[:],
                )
                yp = ppool.tile([P, D], f32, tag="ynp", bufs=2)
                nc.tensor.matmul(
                    out=yp[:], lhsT=xT[:], rhs=Wt["W_rel1"][:], start=True, stop=True
                )
                ys = wpool.tile([P, D], bf, tag="ysb")
                nc.scalar.activation(
                    out=ys[:], in_=yp[:], func=mybir.ActivationFunctionType.Copy
                )
                nc.sync.dma_start(out=y_own[jb * P : (jb + 1) * P, :], in_=ys[:])
                rp = ppool.tile([P, D], f32, tag="rnp", bufs=2)
                nc.tensor.matmul(
                    out=rp[:], lhsT=xT[:], rhs=Wt["W_root1"][:], start=True, stop=True
                )
                nc.vector.tensor_add(
                    out=r_res[:, jb * D : (jb + 1) * D], in0=rp[:], in1=bt["b1"][:]
                )

            nc.gpsimd.collective_compute(
                "AllGather",
                mybir.AluOpType.bypass,
                replica_groups=[list(range(NCORES))],
                ins=[y_own[:].opt()],
                outs=[table2[:].opt()],
            )

            # ---------------- 3 gather/aggregate layers
            for layer in (1, 2, 3):
                W_rel_nxt = Wt[f"W_rel{layer + 1}"] if layer < 3 else None
                W_root_nxt = Wt[f"W_root{layer + 1}"] if layer < 3 else None
                b_nxt = bt[f"b{layer + 1}"] if layer < 3 else None

                for jb in range(B):
                    K = int(K_j[jb])
                    off = int(off_j[jb])
                    ntok = K * P
                    t0 = int(tok_cum[jb])
                    g = gpool.tile([P, K * QD], bf, tag="g")
                    nc.gpsimd.dma_gather(
                        out_ap=g[:].rearrange("p (c e) -> p c e", e=QD),
                        in_ap=table2[:],
                        idxs_ap=idx_res[:, t0 // 16 : (t0 + ntok) // 16],
                        num_idxs=ntok,
                        num_idxs_reg=ntok,
                        elem_size=QD,
                        single_packet=False,
                        queue_num=jb % 4,
                    )
                    # g *= w4 (broadcast along feature dim); slots are 4K wide
                    g3 = g[:].rearrange("p (k f) -> p k f", f=D)
                    wb = (
                        w4[:, 4 * off : 4 * (off + K)]
                        .unsqueeze(-1)
                        .to_broadcast([P, 4 * K, D])
                    )
                    nc.vector.tensor_tensor(
                        out=g3, in0=g3, in1=wb, op=mybir.AluOpType.mult
                    )
                    # agg[p, f] = sum_k g[p, k, f]
                    agg = wpool.tile([P, D], f32, tag="agg")
                    gT = g[:].rearrange("p (k f) -> p f k", f=D)
                    nc.vector.reduce_sum(
                        out=agg[:], in_=gT, axis=mybir.AxisListType.X
                    )

                    # pre = agg + r
                    pre = wpool.tile([P, D], f32, tag="pre")
                    nc.vector.tensor_add(
                        out=pre[:],
                        in0=agg[:],
                        in1=r_res[:, jb * D : (jb + 1) * D],
                    )

                    if layer == 3:
                        # stash pre.T and its per-column |max| partial; the
                        # int8 emit happens after the cross-core max reduce
                        preT = ppool.tile([D, P], f32, tag="preT", bufs=2)
                        nc.tensor.transpose(
                            out=preT[:], in_=pre[:], identity=ident[:]
                        )
                        nc.scalar.activation(
                            out=pre3[:, jb * P : (jb + 1) * P],
                            in_=preT[:],
                            func=mybir.ActivationFunctionType.Copy,
                        )
                        nc.vector.reduce_max(
                            out=cmax[:, jb : jb + 1],
                            in_=pre3[:, jb * P : (jb + 1) * P],
                            axis=mybir.AxisListType.X,
                            apply_absolute_value=True,
                        )
                        continue
                    # hT = relu(pre).T  via PE transpose + ACT evacuation
                    preT = ppool.tile([D, P], f32, tag="preT", bufs=2)
                    nc.tensor.transpose(out=preT[:], in_=pre[:], identity=ident[:])
                    hT = wpool.tile([D, P], f32, tag="hT")
                    nc.scalar.activation(
                        out=hT[:], in_=preT[:], func=mybir.ActivationFunctionType.Relu
                    )
                    # y_next = h @ W_rel (node-major direct: lhsT = hT)
                    ynp = ppool.tile([P, D], f32, tag="ynp", bufs=2)
                    nc.tensor.matmul(
                        out=ynp[:], lhsT=hT[:], rhs=W_rel_nxt[:], start=True, stop=True
                    )
                    ysb = wpool.tile([P, D], bf, tag="ysb")
                    nc.scalar.activation(
                        out=ysb[:], in_=ynp[:], func=mybir.ActivationFunctionType.Copy
                    )
                    nc.sync.dma_start(
                        out=y_own[jb * P : (jb + 1) * P, :], in_=ysb[:]
                    )
                    # r_next = h @ W_root + b (bias via DVE during PSUM evac)
                    rnp = ppool.tile([P, D], f32, tag="rnp", bufs=2)
                    nc.tensor.matmul(
                        out=rnp[:], lhsT=hT[:], rhs=W_root_nxt[:], start=True, stop=True
                    )
                    nc.vector.tensor_add(
                        out=r_res[:, jb * D : (jb + 1) * D],
                        in0=rnp[:],
                        in1=b_nxt[:],
                    )

                if layer < 3:
                    nc.gpsimd.collective_compute(
                        "AllGather",
                        mybir.AluOpType.bypass,
                        replica_groups=[list(range(NCORES))],
                        ins=[y_own[:].opt()],
                        outs=[table2[:].opt()],
                    )

            # ---------------- epilogue: global colmax -> int8 emit
            # cm1 = (per-core colmax)/127; AllReduce-max; emit scale = 1/cm1.
            # The host dequant scale is then exactly the shipped colmax value.
            cm1 = wpool.tile([D, 1], f32, tag="cm1")
            nc.vector.reduce_max(
                out=cm1[:], in_=cmax[:], axis=mybir.AxisListType.X
            )
            nc.vector.tensor_scalar_mul(out=cm1[:], in0=cm1[:], scalar1=1.0 / 127.0)
            nc.sync.dma_start(out=cm_own[:], in_=cm1[:])
            nc.gpsimd.collective_compute(
                "AllReduce",
                mybir.AluOpType.max,
                replica_groups=[list(range(NCORES))],
                ins=[cm_own[:].opt()],
                outs=[cm_red[:].opt()],
            )
            nc.sync.dma_start(out=cmr[:], in_=cm_red[:])
            nc.sync.dma_start(out=cmax_out.ap(), in_=cm_red[:])
            nc.vector.reciprocal(out=scale_res[:], in_=cmr[:])
            for jb in range(B):
                obi = wpool.tile([D, P], i8, tag="obi")
                nc.scalar.activation(
                    out=obi[:],
                    in_=pre3[:, jb * P : (jb + 1) * P],
                    func=mybir.ActivationFunctionType.Copy,
                    scale=scale_res[:],
                )
                nc.sync.dma_start(
                    out=out_t.ap()[:, jb * P : (jb + 1) * P], in_=obi[:]
                )

    nc.compile()
    return nc


# ---------------------------------------------------------------- entry


def _prep_and_build(inputs):
    prep = _preprocess(inputs["x"], inputs["edge_index"], inputs["edge_weight"])
    nc = _build(prep)
    B = prep["B"]
    K = prep["K_total"]

    # pf32 (identical on every core): b1|b2|b3 | W pairs | sx
    pf32 = np.zeros((P, 385), dtype=np.float32)
    for i in (1, 2, 3):
        b = np.asarray(inputs[f"b_rel{i}"], dtype=np.float32)
        pf32[:, (i - 1) * D : i * D] = b[None, :]
        pf32[0:D, 192 + (i - 1) * D : 192 + i * D] = np.asarray(
            inputs[f"W_rel{i}"], dtype=np.float32
        )
        pf32[D : 2 * D, 192 + (i - 1) * D : 192 + i * D] = np.asarray(
            inputs[f"W_root{i}"], dtype=np.float32
        )
    pf32[0:D, 384:385] = prep["sx"]

    in_maps = []
    for c in range(NCORES):
        x_pb = (
            prep["x_arr"][c].reshape(B, P, D).transpose(1, 0, 2).reshape(P, B * D)
        )
        pi8 = np.concatenate([x_pb, prep["sel_pk"][c]], axis=1)
        toks = prep["idx_tok"][c].reshape(-1).reshape(P, K)
        pi16 = np.concatenate([toks, prep["ell_w"][c].view(np.int16)], axis=1)
        in_maps.append(
            {
                "pi8": np.ascontiguousarray(pi8),
                "pi16": np.ascontiguousarray(pi16),
                "pf32": pf32 if c == 0 else np.zeros_like(pf32),
            }
        )
    return prep, nc, in_maps


def _reassemble(prep, core_outs, core_cmax):
    N = prep["N"]
    B = prep["B"]
    perm = prep["perm"]
    out = np.zeros((N, D), dtype=np.float32)
    for c in range(NCORES):
        # h3 arrives transposed [D, B*P] int8; colmax is already the
        # dequant scale (global |max|/127)
        sc = core_cmax[c].reshape(D).astype(np.float32)
        h = core_outs[c].astype(np.float32).T * sc[None, :]
        pr = perm[c * B * P : (c + 1) * B * P]
        real = pr >= 0
        out[pr[real]] = h[real]
    return out


def kernel(**inputs) -> np.ndarray:
    from concourse.bass_utils import run_bass_kernel_spmd

    prep, nc, in_maps = _prep_and_build(inputs)
    res = run_bass_kernel_spmd(
        nc,
        in_maps,
        core_ids=list(range(NCORES)),
        trace=bool(int(os.environ.get("GCN_TRACE", "0"))),
    )
    kernel.last_results = res
    kernel.last_nc = nc
    kernel.last_in_maps = in_maps
    return _reassemble(
        prep,
        [res.results[c]["h3"] for c in range(NCORES)],
        [res.results[c]["colmax"] for c in range(NCORES)],
    )


if __name__ == "__main__":
    import reference

    inputs = {k: np.asarray(v) for k, v in reference.setup_inputs().items()}
    expected = np.asarray(reference.reference(**inputs))
    actual = kernel(**inputs)
    err = np.abs(actual - expected).max() / (np.abs(expected).max() + 1e-9)
    rel = np.linalg.norm(actual - expected) / (np.linalg.norm(expected) + 1e-30)
    print("max-abs-rel:", err, " fro-rel:", rel)
